# revision 7
# baseline (speedup 1.0000x reference)
"""Trainium2 Bass kernel for nn_DMS_STGAT (dual-branch GAT attention softmaxes).

Strategy (per core, data-parallel over batch B=16 -> 2 per core):
  The reference only uses h = x @ W through two dots s1 = h@a[:F], s2 = h@a[F:],
  so  e[bt, n1, n2] = LRelu(s1[r1[m]] + s2[r2[m]])  with fixed index maps r1/r2
  (the "scrambled pairing").  s1/s2/t1/t2 are 128-dim dots against
  host-precomputed Wa = W@a vectors; E is assembled by a stacked-K PE matmul
  against tiny host-precomputed 0/1 matrices; the double softmax runs on-chip.
  Spatial PE term exp(-||x_j - x_8||/1000) rides extra stacked rows with
  Qs = S1*Q1 + S2*Q2; temporal positional constant qp rides a ones-row;
  exp-overflow safety comes from a post-LRelu per-group constant shift.

  v3: wa-stationary dot matmuls (wa4 is the 128x2 stationary operand, X
  streams as the moving operand) into psum rows {0,32,64}; X is loaded in
  (t,b,j) column order and X_jt is built in (j,b,t) order so every
  stacked-K gather is a single contiguous-row -> [25,50] reshape DMA and
  both outputs are contiguous DMAs.  LeakyReLU is one Prelu ACT op.
"""
import sys
import numpy as np

for _p in ("/opt/trn_rl_repo", "/root/.axon_site/_ro/trn_rl_repo"):
    if _p not in sys.path:
        sys.path.insert(0, _p)

from contextlib import ExitStack  # noqa: E402

import concourse.bass as bass  # noqa: E402
import concourse.tile as tile  # noqa: E402
from concourse import bacc, mybir  # noqa: E402

B, C, T, J, F = 16, 128, 25, 25, 256
N = 25            # N == T == J
NN = N * N        # 625
NL = 2 * NN       # 1250 columns per layout
NCORES = 8
BL = B // NCORES  # 2 batches per core
FP = mybir.dt.float32
BF = mybir.dt.bfloat16
AF = mybir.ActivationFunctionType
ALU = mybir.AluOpType

KS = 89           # spatial stack: 0:25 s1, 25:50 s2, 50:64 zero, 64:89 ec
KT = 57           # temporal stack: 0:25 t1, 25 ones, 26:32 zero, 32:57 t2

# n2-split for softmax-chain pipelining
N2SPLITS = [(0, 13), (13, 25)]

# Pin ALL activation functions to one table set (exp/ln/square/prelu/copy live
# together in natural_log_exp_and_others) so only one ACT_TABLE_LOAD happens.
_orig_get_tables = bacc.get_activation_tables


def _pinned_tables(arch):
    tabs = dict(_orig_get_tables(arch))
    assert "natural_log_exp_and_others" in tabs
    return {k: (v if k == "natural_log_exp_and_others" else set())
            for k, v in tabs.items()}


bacc.get_activation_tables = _pinned_tables

# ---------------------------------------------------------------- host math --

def _pair_indices():
    r1 = np.zeros(NN, np.int64)
    r2 = np.zeros(NN, np.int64)
    for m in range(NN):
        k1, k2 = 2 * m, 2 * m + 1
        r1[m] = (k1 // N) if k1 < NN else ((k1 - NN) % N)
        r2[m] = (k2 // N) if k2 < NN else ((k2 - NN) % N)
    return r1, r2


def _sinusoid_pos():
    pos = np.arange(200)[:, None].astype(np.float64)
    hid = np.arange(C)[None, :]
    angle = pos / np.power(10000.0, 2.0 * (hid // 2) / C)
    tab = angle.copy()
    tab[:, 0::2] = np.sin(angle[:, 0::2])
    tab[:, 1::2] = np.cos(angle[:, 1::2])
    return tab[:T] * 1000.0  # [T, C] float64


_R1, _R2 = _pair_indices()


def _host_consts(W_s, a_s, W_t, a_t):
    """Precompute tiny derived params in float64. ~0.3 MFLOP."""
    W_s = W_s.astype(np.float64)
    a_s = a_s.astype(np.float64)
    W_t = W_t.astype(np.float64)
    a_t = a_t.astype(np.float64)
    wa_s1 = W_s @ a_s[:F, 0]
    wa_s2 = W_s @ a_s[F:, 0]
    wa_t1 = W_t @ a_t[:F, 0]
    wa_t2 = W_t @ a_t[F:, 0]
    S1, S2 = wa_s1.sum(), wa_s2.sum()

    Q1 = np.zeros((N, NN), np.float64)
    Q2 = np.zeros((N, NN), np.float64)
    Q1[_R1, np.arange(NN)] = 1.0
    Q2[_R2, np.arange(NN)] = 1.0
    qs = S1 * Q1 + S2 * Q2

    pos = _sinusoid_pos()
    p1 = pos @ wa_t1
    p2 = pos @ wa_t2
    qp = p1[_R1] + p2[_R2]
    qLR = np.where(qp > 0, qp, 0.2 * qp)
    cq = qLR.reshape(N, N).max(axis=0)

    wa4 = np.stack([wa_s1, wa_s2, wa_t1, wa_t2], axis=1)  # [128, 4]
    # permute the m-axis to n2-major (m' = n2*25 + n1) so the softmax chain
    # and its n1-group reductions are contiguous on-chip
    mperm = (np.arange(NN) % N) * N + (np.arange(NN) // N)  # m' -> orig m
    qstk_s = np.zeros((KS, NN), np.float64)
    qstk_s[0:N] = Q1[:, mperm]
    qstk_s[N:2 * N] = Q2[:, mperm]
    qstk_s[64:64 + N] = qs[:, mperm]
    qstk_t = np.zeros((KT, NN), np.float64)
    qstk_t[0:N] = Q1[:, mperm]
    qstk_t[N] = qp[mperm]
    qstk_t[32:32 + N] = Q2[:, mperm]
    csh = cq[np.arange(NN) // N][None, :]       # n2-major
    return (wa4.astype(np.float32), qstk_s.astype(np.float32),
            qstk_t.astype(np.float32), csh.astype(np.float32))


# ------------------------------------------------------------- bass program --

def _build_program():
    nc = bacc.Bacc("TRN2", target_bir_lowering=False, debug=False)

    src_d = nc.dram_tensor("src_l", [BL, C, T, J], FP, kind="ExternalInput").ap()
    wa4_d = nc.dram_tensor("wa4", [C, 4], FP, kind="ExternalInput").ap()
    qss_d = nc.dram_tensor("qstk_s", [KS, NN], FP, kind="ExternalInput").ap()
    qst_d = nc.dram_tensor("qstk_t", [KT, NN], FP, kind="ExternalInput").ap()
    csh_d = nc.dram_tensor("csh", [1, NN], FP, kind="ExternalInput").ap()
    outs_d = nc.dram_tensor("out_s", [BL, T, N, N], FP, kind="ExternalOutput").ap()
    outt_d = nc.dram_tensor("out_t", [BL, T, N, N], FP, kind="ExternalOutput").ap()

    with tile.TileContext(nc) as tc, ExitStack() as ctx:
        consts = ctx.enter_context(tc.tile_pool(name="consts", bufs=1))
        data = ctx.enter_context(tc.tile_pool(name="data", bufs=1))
        pp = ctx.enter_context(tc.tile_pool(name="pp", bufs=1, space="PSUM"))

        # --- input: X[c, t*50 + b*25 + j] (t, b, j) col order; 2 DMAs ---
        X = data.tile([C, NL], FP)
        FX = X[:].ap[0][0]
        for b in range(BL):
            src_b = bass.AP(tensor=src_d.tensor, offset=src_d.offset + b * C * NN,
                            ap=[[NN, C], [N, N], [1, N]])       # (c, t, j)
            dst_b = bass.AP(tensor=X.tensor, offset=X.offset + b * N,
                            ap=[[FX, C], [2 * N, N], [1, N]])
            nc.sync.dma_start(dst_b, src_b)

        wa4 = consts.tile([C, 4], FP)
        nc.scalar.dma_start(wa4[:], wa4_d)

        # --- ACT table warm-up (table load is auto-inserted before this) ---
        dummy = consts.tile([1, 2], FP)
        nc.vector.memset(dummy[:], 0.0)
        nc.scalar.activation(dummy[:], dummy[:], AF.Exp)

        # --- dep-free memsets on DVE ---
        ones_bf = consts.tile([C, 1], BF)
        nc.vector.memset(ones_bf[:], 1.0)
        SPK = data.tile([KS, 50], FP)
        nc.vector.memset(SPK[:], 0.0)
        TPK = data.tile([KT, 50], FP)
        nc.vector.memset(TPK[:], 0.0)
        onesrow = consts.tile([1, 50], FP)
        nc.vector.memset(onesrow[:], 1.0)
        nc.sync.dma_start(TPK[N:N + 1, :], onesrow[:])   # temporal ones row
        eps_b = consts.tile([89, 1], FP)
        nc.vector.memset(eps_b[:], 1e-30)

        # --- big consts on the ACT ring (idle DMA time) ---
        qst = consts.tile([KT, NN], FP)
        nc.scalar.dma_start(qst[:], qst_d)
        qss = consts.tile([KS, NN], FP)
        nc.scalar.dma_start(qss[:], qss_d)
        CSHt = consts.tile([114, NN], FP)
        csh_b = bass.AP(tensor=csh_d.tensor, offset=csh_d.offset,
                        ap=[[0, 50], [1, NN]])
        nc.scalar.dma_start(CSHt[64:114, :], csh_b)

        # --- X_jt[c, j*50 + b*25 + t] (j, b, t) col order ---
        X_jt = data.tile([C, NL], FP)
        FXJ = X_jt[:].ap[0][0]
        for b, eng in ((0, nc.scalar), (1, nc.gpsimd)):
            xin = bass.AP(tensor=X.tensor, offset=X.offset + b * N,
                          ap=[[FX, C], [1, N], [2 * N, N]])     # (c, j, t)
            xout = bass.AP(tensor=X_jt.tensor, offset=X_jt.offset + b * N,
                           ap=[[FXJ, C], [2 * N, N], [1, N]])
            if b == 0:
                eng.copy(xout, xin)
            else:
                eng.tensor_copy(xout, xin)

        # --- temporal dots first: psum rows 32:34 = {t1, t2} over X chunks ---
        psum_d = pp.tile([66, NL], FP)
        for lo, hi in ((0, 512), (512, 1024), (1024, NL)):
            nc.tensor.matmul(psum_d[32:34, lo:hi], wa4[:, 2:4], X[:, lo:hi],
                             start=True, stop=True)

        # --- D = X_jt - ref_j8 (bf16), D2 = D^2 (bf16), per b ---
        D = data.tile([C, NL], BF)
        D2 = data.tile([C, NL], BF)
        FD = D[:].ap[0][0]
        for b, eng in ((0, nc.vector), (1, nc.gpsimd)):
            in0 = bass.AP(tensor=X_jt.tensor, offset=X_jt.offset + b * N,
                          ap=[[FXJ, C], [2 * N, N], [1, N]])
            ref = bass.AP(tensor=X_jt.tensor, offset=X_jt.offset + 8 * 2 * N + b * N,
                          ap=[[FXJ, C], [0, N], [1, N]])
            dout = bass.AP(tensor=D.tensor, offset=D.offset + b * N,
                           ap=[[FD, C], [2 * N, N], [1, N]])
            eng.tensor_tensor(dout, in0, ref, op=ALU.subtract)
        nc.scalar.activation(D2[:, 0:NN], D[:, 0:NN], AF.Square)
        nc.gpsimd.tensor_tensor(D2[:, NN:NL], D[:, NN:NL], D[:, NN:NL],
                                op=ALU.mult)

        # --- spatial dots: psum rows 0:2 = {s1, s2} over X_jt chunks ---
        for lo, hi in ((0, 512), (512, 1024), (1024, NL)):
            nc.tensor.matmul(psum_d[0:2, lo:hi], wa4[:, 0:2], X_jt[:, lo:hi],
                             start=True, stop=True)
        # --- d2 sums: psum row 64 over D2 chunks (bank-aligned outs) ---
        for lo, hi in ((0, 512), (512, 1024), (1024, NL)):
            nc.tensor.matmul(psum_d[64:65, lo:hi], ones_bf[:], D2[:, lo:hi],
                             start=True, stop=True)

        # --- stage dot rows to SBUF (col halves on DVE + Pool) ---
        S5 = data.tile([66, NL], FP)
        FS5 = S5[:].ap[0][0]
        nc.vector.tensor_copy(S5[:, 0:NN], psum_d[:, 0:NN])
        nc.scalar.copy(S5[:, NN:NL], psum_d[:, NN:NL])

        # --- scatters: contiguous row -> [25, 50] reshape DMAs ---
        def scatter(row, dst_t, rbase, eng):
            fdst = dst_t[:].ap[0][0]
            src = bass.AP(tensor=S5.tensor, offset=S5.offset + row * FS5,
                          ap=[[FS5, 1], [1, NL]])
            dst = bass.AP(tensor=dst_t.tensor, offset=dst_t.offset + rbase * fdst,
                          ap=[[fdst, N], [1, 50]])
            eng.dma_start(dst, src)

        scatter(32, TPK, 0, nc.sync)     # t1
        scatter(33, TPK, 32, nc.scalar)  # t2
        scatter(0, SPK, 0, nc.sync)      # s1
        scatter(1, SPK, 25, nc.scalar)   # s2
        scatter(64, SPK, 64, nc.sync)    # d2 sums

        # --- EC = exp(-sqrt(d2)/1000) via exp(0.5*ln) on SPK rows 64:89 ---
        ecL = data.tile([89, 50], FP)
        nc.scalar.activation(ecL[64:89, :], SPK[64:89, 0:50], AF.Ln,
                             bias=eps_b[64:89])
        ecW = data.tile([89, 50], FP)
        nc.scalar.activation(ecW[64:89, :], ecL[64:89, :], AF.Exp, scale=0.5)
        nc.scalar.activation(SPK[64:89, 0:50], ecW[64:89, :], AF.Exp, scale=-0.001)

        # --- E matmuls (stacked-K): spatial rows 0:50, temporal 64:114 ---
        psum_E = pp.tile([114, 1024], FP)
        nc.vector.memset(psum_E[32:64, 0:NN], 0.0)  # junk rows 50:64 stay finite
        chunks = [(0, 512), (512, NN)]
        for lo, hi in chunks:
            nc.tensor.matmul(psum_E[64:114, lo:hi], TPK[:, :], qst[:, lo:hi],
                             start=True, stop=True, tile_position=(0, 64))
            nc.tensor.matmul(psum_E[0:50, lo:hi], SPK[:, :], qss[:, lo:hi],
                             start=True, stop=True)

        # --- softmax tail (m is n2-major: groups are contiguous 25-runs) ---
        E2 = data.tile([114, NN], FP)
        g = data.tile([114, NN], FP)
        Z = data.tile([114, N], FP)
        Zr = data.tile([114, N], FP)
        att1 = data.tile([114, NN], FP)
        g2 = data.tile([114, NN], FP)
        Z2 = data.tile([114, N], FP)
        Z2r = data.tile([114, N], FP)
        outF = data.tile([114, NN], FP)
        FO = outF[:].ap[0][0]

        def gview(t, lo, hi, npart=114, p0=0):
            """[(p), (n2 groups), (n1 contiguous)] view."""
            fs = t[:].ap[0][0]
            return bass.AP(tensor=t.tensor, offset=t.offset + p0 * fs + lo * N,
                           ap=[[fs, npart], [N, hi - lo], [1, N]])

        def bview(t, lo, hi, npart=114, p0=0):
            """broadcast [(p), (n2), (n1 step-0)] view of a [*, 25] tile."""
            fs = t[:].ap[0][0]
            return bass.AP(tensor=t.tensor, offset=t.offset + p0 * fs + lo,
                           ap=[[fs, npart], [1, hi - lo], [0, N]])

        for lo, hi in N2SPLITS:
            cl, ch = lo * N, hi * N
            # LeakyReLU in one ACT op; temporal rows then get -csh
            nc.scalar.activation(E2[:, cl:ch], psum_E[:, cl:ch], AF.Prelu,
                                 alpha=0.2)
            nc.gpsimd.tensor_tensor(E2[64:114, cl:ch], E2[64:114, cl:ch],
                                    CSHt[64:114, cl:ch], op=ALU.subtract)
            # softmax 1
            nc.scalar.activation(g[:, cl:ch], E2[:, cl:ch], AF.Exp)
            nc.vector.tensor_reduce(Z[:, lo:hi], gview(g, lo, hi),
                                    axis=mybir.AxisListType.X, op=ALU.add)
            nc.vector.reciprocal(Zr[:, lo:hi], Z[:, lo:hi])
            nc.gpsimd.tensor_tensor(gview(att1, lo, hi), gview(g, lo, hi),
                                    bview(Zr, lo, hi), op=ALU.mult)
            # softmax 2
            nc.scalar.activation(g2[:, cl:ch], att1[:, cl:ch], AF.Exp)
            nc.vector.tensor_reduce(Z2[:, lo:hi], gview(g2, lo, hi),
                                    axis=mybir.AxisListType.X, op=ALU.add)
            nc.vector.reciprocal(Z2r[:, lo:hi], Z2[:, lo:hi])
            # final scale, writing transposed back to n1-major for output
            oswap = bass.AP(tensor=outF.tensor, offset=outF.offset + lo,
                            ap=[[FO, 114], [1, hi - lo], [N, N]])
            nc.gpsimd.tensor_tensor(oswap, gview(g2, lo, hi),
                                    bview(Z2r, lo, hi), op=ALU.mult)

            # outputs for this n2-group: cols {n1*25 + g} of rows (b, t)/(b, j)
            for (p0, od, eng) in ((0, outs_d, nc.sync), (64, outt_d, nc.scalar)):
                srcv = bass.AP(tensor=outF.tensor,
                               offset=outF.offset + p0 * FO + lo,
                               ap=[[FO, 50], [N, N], [1, hi - lo]])
                dstv = bass.AP(tensor=od.tensor, offset=od.offset + lo,
                               ap=[[NN, 50], [N, N], [1, hi - lo]])
                eng.dma_start(dstv, srcv)

    nc.compile()
    return nc


_PROGRAM = None


def _get_program():
    global _PROGRAM
    if _PROGRAM is None:
        _PROGRAM = _build_program()
    return _PROGRAM


# ------------------------------------------------------------------ kernel --

def kernel(src, W_s, a_s, W_t, a_t):
    from concourse.bass_utils import run_bass_kernel_spmd

    src = np.ascontiguousarray(np.asarray(src, dtype=np.float32))
    wa4, qstk_s, qstk_t, csh = _host_consts(np.asarray(W_s), np.asarray(a_s),
                                            np.asarray(W_t), np.asarray(a_t))
    nc = _get_program()
    in_maps = []
    for c in range(NCORES):
        in_maps.append({
            "src_l": src[c * BL:(c + 1) * BL],
            "wa4": wa4, "qstk_s": qstk_s, "qstk_t": qstk_t, "csh": csh,
        })
    res = run_bass_kernel_spmd(nc, in_maps, core_ids=list(range(NCORES)))
    out_s = np.concatenate([res.results[c]["out_s"] for c in range(NCORES)], axis=0)
    out_t = np.concatenate([res.results[c]["out_t"] for c in range(NCORES)], axis=0)
    return out_s, out_t


# revision 10
# speedup vs baseline: 1.2522x; 1.2522x over previous
"""Trainium2 Bass kernel for nn_DMS_STGAT (dual-branch GAT attention softmaxes).

Strategy (per core, data-parallel over batch B=16 -> 2 per core):
  The reference only uses h = x @ W through two dots s1 = h@a[:F], s2 = h@a[F:],
  so  e[bt, n1, n2] = LRelu(s1[r1[m]] + s2[r2[m]])  with fixed index maps r1/r2
  (the "scrambled pairing").  s1/s2/t1/t2 are 128-dim dots against
  host-precomputed Wa = W@a vectors; E is assembled by a stacked-K PE matmul
  against tiny host-precomputed 0/1 matrices; the double softmax runs on-chip.
  Spatial PE term exp(-||x_j - x_8||/1000) rides extra stacked rows with
  Qs = S1*Q1 + S2*Q2; temporal positional constant qp rides a ones-row;
  exp-overflow safety comes from a post-LRelu per-group constant shift.

  v3: wa-stationary dot matmuls (wa4 is the 128x2 stationary operand, X
  streams as the moving operand) into psum rows {0,32,64}; X is loaded in
  (t,b,j) column order and X_jt is built in (j,b,t) order so every
  stacked-K gather is a single contiguous-row -> [25,50] reshape DMA and
  both outputs are contiguous DMAs.  LeakyReLU is one Prelu ACT op.
"""
import sys
import numpy as np

for _p in ("/opt/trn_rl_repo", "/root/.axon_site/_ro/trn_rl_repo"):
    if _p not in sys.path:
        sys.path.insert(0, _p)

from contextlib import ExitStack  # noqa: E402

import concourse.bass as bass  # noqa: E402
import concourse.tile as tile  # noqa: E402
from concourse import bacc, mybir  # noqa: E402

B, C, T, J, F = 16, 128, 25, 25, 256
N = 25            # N == T == J
NN = N * N        # 625
NL = 2 * NN       # 1250 columns per layout
NCORES = 8
BL = B // NCORES  # 2 batches per core
FP = mybir.dt.float32
BF = mybir.dt.bfloat16
AF = mybir.ActivationFunctionType
ALU = mybir.AluOpType

KS = 89           # spatial stack: 0:25 s1, 25:50 s2, 50:64 zero, 64:89 ec
KT = 57           # temporal stack: 0:25 t1, 25 ones, 26:32 zero, 32:57 t2

# n2-split for softmax-chain pipelining
N2SPLITS = [(0, 13), (13, 25)]

# Pin ALL activation functions to one table set (exp/ln/square/prelu/copy live
# together in natural_log_exp_and_others) so only one ACT_TABLE_LOAD happens.
_orig_get_tables = bacc.get_activation_tables


def _pinned_tables(arch):
    tabs = dict(_orig_get_tables(arch))
    assert "natural_log_exp_and_others" in tabs
    return {k: (v if k == "natural_log_exp_and_others" else set())
            for k, v in tabs.items()}


bacc.get_activation_tables = _pinned_tables

# ---------------------------------------------------------------- host math --

def _pair_indices():
    r1 = np.zeros(NN, np.int64)
    r2 = np.zeros(NN, np.int64)
    for m in range(NN):
        k1, k2 = 2 * m, 2 * m + 1
        r1[m] = (k1 // N) if k1 < NN else ((k1 - NN) % N)
        r2[m] = (k2 // N) if k2 < NN else ((k2 - NN) % N)
    return r1, r2


def _sinusoid_pos():
    pos = np.arange(200)[:, None].astype(np.float64)
    hid = np.arange(C)[None, :]
    angle = pos / np.power(10000.0, 2.0 * (hid // 2) / C)
    tab = angle.copy()
    tab[:, 0::2] = np.sin(angle[:, 0::2])
    tab[:, 1::2] = np.cos(angle[:, 1::2])
    return tab[:T] * 1000.0  # [T, C] float64


_R1, _R2 = _pair_indices()


def _host_consts(W_s, a_s, W_t, a_t):
    """Precompute tiny derived params in float64. ~0.3 MFLOP."""
    W_s = W_s.astype(np.float64)
    a_s = a_s.astype(np.float64)
    W_t = W_t.astype(np.float64)
    a_t = a_t.astype(np.float64)
    wa_s1 = W_s @ a_s[:F, 0]
    wa_s2 = W_s @ a_s[F:, 0]
    wa_t1 = W_t @ a_t[:F, 0]
    wa_t2 = W_t @ a_t[F:, 0]
    S1, S2 = wa_s1.sum(), wa_s2.sum()

    Q1 = np.zeros((N, NN), np.float64)
    Q2 = np.zeros((N, NN), np.float64)
    Q1[_R1, np.arange(NN)] = 1.0
    Q2[_R2, np.arange(NN)] = 1.0
    qs = S1 * Q1 + S2 * Q2

    pos = _sinusoid_pos()
    p1 = pos @ wa_t1
    p2 = pos @ wa_t2
    qp = p1[_R1] + p2[_R2]
    qLR = np.where(qp > 0, qp, 0.2 * qp)
    cq = qLR.reshape(N, N).max(axis=0)

    wa4 = np.stack([wa_s1, wa_s2, wa_t1, wa_t2], axis=1)  # [128, 4]
    # permute the m-axis to n2-major (m' = n2*25 + n1) so the softmax chain
    # and its n1-group reductions are contiguous on-chip
    mperm = (np.arange(NN) % N) * N + (np.arange(NN) // N)  # m' -> orig m
    qstk_s = np.zeros((KS, NN), np.float64)
    qstk_s[0:N] = Q1[:, mperm]
    qstk_s[N:2 * N] = Q2[:, mperm]
    qstk_s[64:64 + N] = qs[:, mperm]
    qstk_t = np.zeros((KT, NN), np.float64)
    qstk_t[0:N] = Q1[:, mperm]
    qstk_t[N] = qp[mperm]
    qstk_t[32:32 + N] = Q2[:, mperm]
    csh = np.broadcast_to(cq[np.arange(NN) // N][None, :], (50, NN)).copy()
    return (wa4.astype(np.float32), qstk_s.astype(np.float32),
            qstk_t.astype(np.float32), csh.astype(np.float32))


# ------------------------------------------------------------- bass program --

def _build_program():
    nc = bacc.Bacc("TRN2", target_bir_lowering=False, debug=False)

    src_d = nc.dram_tensor("src_l", [BL, C, T, J], FP, kind="ExternalInput").ap()
    wa4_d = nc.dram_tensor("wa4", [C, 4], FP, kind="ExternalInput").ap()
    qss_d = nc.dram_tensor("qstk_s", [KS, NN], FP, kind="ExternalInput").ap()
    qst_d = nc.dram_tensor("qstk_t", [KT, NN], FP, kind="ExternalInput").ap()
    csh_d = nc.dram_tensor("csh", [50, NN], FP, kind="ExternalInput").ap()
    outs_d = nc.dram_tensor("out_s", [BL, T, N, N], FP, kind="ExternalOutput").ap()
    outt_d = nc.dram_tensor("out_t", [BL, T, N, N], FP, kind="ExternalOutput").ap()

    with tile.TileContext(nc) as tc, ExitStack() as ctx:
        consts = ctx.enter_context(tc.tile_pool(name="consts", bufs=1))
        data = ctx.enter_context(tc.tile_pool(name="data", bufs=1))
        pp = ctx.enter_context(tc.tile_pool(name="pp", bufs=1, space="PSUM"))

        # --- input: X[c, b*625 + t*25 + j] (b, t, j); contiguous 2 DMAs ---
        X = data.tile([C, NL], FP)
        FX = X[:].ap[0][0]
        for b in range(BL):
            src_b = bass.AP(tensor=src_d.tensor, offset=src_d.offset + b * C * NN,
                            ap=[[NN, C], [1, NN]])
            nc.sync.dma_start(X[:, b * NN:(b + 1) * NN], src_b)

        wa4 = consts.tile([C, 4], FP)
        nc.scalar.dma_start(wa4[:], wa4_d)
        onesrow = consts.tile([1, 50], FP)

        # --- ACT table warm-up (table load is auto-inserted before this) ---
        dummy = consts.tile([1, 2], FP)
        nc.vector.memset(dummy[:], 0.0)
        nc.scalar.activation(dummy[:], dummy[:], AF.Exp)

        # --- dep-free memsets on DVE ---
        ones_bf = consts.tile([C, 1], BF)
        nc.vector.memset(ones_bf[:], 1.0)
        SPK = data.tile([KS, 50], FP)
        nc.vector.memset(SPK[:], 0.0)
        TPK = data.tile([KT, 50], FP)
        nc.vector.memset(TPK[:], 0.0)
        nc.vector.memset(onesrow[:], 1.0)
        nc.sync.dma_start(TPK[N:N + 1, :], onesrow[:])   # temporal ones row
        eps_b = consts.tile([89, 1], FP)
        nc.vector.memset(eps_b[:], 1e-30)

        # --- big consts on the sync ring (idle until the scatters) ---
        qst = consts.tile([KT, NN], FP)
        nc.sync.dma_start(qst[:], qst_d)
        qss = consts.tile([KS, NN], FP)
        nc.sync.dma_start(qss[:], qss_d)
        CSHt = consts.tile([114, NN], FP)
        nc.sync.dma_start(CSHt[64:114, :], csh_d)

        # --- X_jt[c, j*50 + b*25 + t] (j, b, t) col order ---
        X_jt = data.tile([C, NL], FP)
        FXJ = X_jt[:].ap[0][0]
        for b, eng in ((0, nc.scalar), (1, nc.vector)):
            xin = bass.AP(tensor=X.tensor, offset=X.offset + b * NN,
                          ap=[[FX, C], [1, N], [N, N]])         # (c, j, t)
            xout = bass.AP(tensor=X_jt.tensor, offset=X_jt.offset + b * N,
                           ap=[[FXJ, C], [2 * N, N], [1, N]])
            if b == 0:
                eng.copy(xout, xin)
            else:
                eng.tensor_copy(xout, xin)

        # --- temporal dots: psum rows 32:34 = {t1, t2}; the moving operand
        #     reads X through a (t, b, j)-ordered strided view so psum cols
        #     land t-major for the contiguous scatter ---
        psum_d = pp.tile([66, NL], FP)
        for tl, th in ((0, 10), (10, 20), (20, 25)):
            xmov = bass.AP(tensor=X.tensor, offset=X.offset + tl * N,
                           ap=[[FX, C], [N, th - tl], [NN, BL], [1, N]])
            nc.tensor.matmul(psum_d[32:34, tl * 50:th * 50], wa4[:, 2:4], xmov,
                             start=True, stop=True)

        # --- D = X_jt - ref_j8 (bf16), D2 = D^2 (bf16), per b ---
        D = data.tile([C, NL], BF)
        D2 = data.tile([C, NL], BF)
        FD = D[:].ap[0][0]
        for b, eng in ((0, nc.gpsimd), (1, nc.vector)):
            in0 = bass.AP(tensor=X_jt.tensor, offset=X_jt.offset + b * N,
                          ap=[[FXJ, C], [2 * N, N], [1, N]])
            ref = bass.AP(tensor=X_jt.tensor, offset=X_jt.offset + 8 * 2 * N + b * N,
                          ap=[[FXJ, C], [0, N], [1, N]])
            dout = bass.AP(tensor=D.tensor, offset=D.offset + b * N,
                           ap=[[FD, C], [2 * N, N], [1, N]])
            eng.tensor_tensor(dout, in0, ref, op=ALU.subtract)
        nc.scalar.activation(D2[:, 0:NN], D[:, 0:NN], AF.Square)
        nc.gpsimd.tensor_tensor(D2[:, NN:NL], D[:, NN:NL], D[:, NN:NL],
                                op=ALU.mult)

        # --- spatial dots: psum rows 0:2 = {s1, s2} over X_jt chunks ---
        for lo, hi in ((0, 512), (512, 1024), (1024, NL)):
            nc.tensor.matmul(psum_d[0:2, lo:hi], wa4[:, 0:2], X_jt[:, lo:hi],
                             start=True, stop=True)
        # --- d2 sums: psum row 64 over D2 chunks (bank-aligned outs) ---
        for lo, hi in ((0, 512), (512, 1024), (1024, NL)):
            nc.tensor.matmul(psum_d[64:65, lo:hi], ones_bf[:], D2[:, lo:hi],
                             start=True, stop=True)

        # --- stage dot rows to SBUF (col halves on DVE + Pool) ---
        S5 = data.tile([66, NL], FP)
        FS5 = S5[:].ap[0][0]
        nc.vector.tensor_copy(S5[:, 0:NN], psum_d[:, 0:NN])
        nc.scalar.copy(S5[:, NN:NL], psum_d[:, NN:NL])

        # --- scatters: contiguous row -> [25, 50] reshape DMAs ---
        def scatter(row, dst_t, rbase, eng):
            fdst = dst_t[:].ap[0][0]
            src = bass.AP(tensor=S5.tensor, offset=S5.offset + row * FS5,
                          ap=[[FS5, 1], [1, NL]])
            dst = bass.AP(tensor=dst_t.tensor, offset=dst_t.offset + rbase * fdst,
                          ap=[[fdst, N], [1, 50]])
            eng.dma_start(dst, src)

        scatter(32, TPK, 0, nc.sync)     # t1
        scatter(33, TPK, 32, nc.scalar)  # t2
        scatter(0, SPK, 0, nc.sync)      # s1
        scatter(1, SPK, 25, nc.scalar)   # s2
        scatter(64, SPK, 64, nc.sync)    # d2 sums

        # --- EC = exp(-sqrt(d2)/1000) via exp(0.5*ln) on SPK rows 64:89 ---
        ecL = data.tile([89, 50], FP)
        nc.scalar.activation(ecL[64:89, :], SPK[64:89, 0:50], AF.Ln,
                             bias=eps_b[64:89])
        ecW = data.tile([89, 50], FP)
        nc.scalar.activation(ecW[64:89, :], ecL[64:89, :], AF.Exp, scale=0.5)
        nc.scalar.activation(SPK[64:89, 0:50], ecW[64:89, :], AF.Exp, scale=-0.001)

        # --- E matmuls (stacked-K): spatial rows 0:50, temporal 64:114 ---
        psum_E = pp.tile([114, 1024], FP)
        nc.vector.memset(psum_E[32:64, 0:NN], 0.0)  # junk rows 50:64 stay finite
        chunks = [(0, 512), (512, NN)]
        for lo, hi in chunks:
            nc.tensor.matmul(psum_E[64:114, lo:hi], TPK[:, :], qst[:, lo:hi],
                             start=True, stop=True, tile_position=(0, 64))
            nc.tensor.matmul(psum_E[0:50, lo:hi], SPK[:, :], qss[:, lo:hi],
                             start=True, stop=True)

        # --- softmax tail (m is n2-major: groups are contiguous 25-runs) ---
        E2 = data.tile([114, NN], FP)
        g = data.tile([114, NN], FP)
        Z = data.tile([114, N], FP)
        Zr = data.tile([114, N], FP)
        att1 = data.tile([114, NN], FP)
        g2 = data.tile([114, NN], FP)
        Z2 = data.tile([114, N], FP)
        Z2r = data.tile([114, N], FP)
        outF = data.tile([114, NN], FP)
        FO = outF[:].ap[0][0]

        def gview(t, lo, hi, npart=114, p0=0):
            """[(p), (n2 groups), (n1 contiguous)] view."""
            fs = t[:].ap[0][0]
            return bass.AP(tensor=t.tensor, offset=t.offset + p0 * fs + lo * N,
                           ap=[[fs, npart], [N, hi - lo], [1, N]])

        def bview(t, lo, hi, npart=114, p0=0):
            """broadcast [(p), (n2), (n1 step-0)] view of a [*, 25] tile."""
            fs = t[:].ap[0][0]
            return bass.AP(tensor=t.tensor, offset=t.offset + p0 * fs + lo,
                           ap=[[fs, npart], [1, hi - lo], [0, N]])

        for lo, hi in N2SPLITS:
            cl, ch = lo * N, hi * N
            # LeakyReLU in one ACT op; temporal rows then get -csh
            nc.scalar.activation(E2[:, cl:ch], psum_E[:, cl:ch], AF.Prelu,
                                 alpha=0.2)
            nc.gpsimd.tensor_tensor(E2[64:114, cl:ch], E2[64:114, cl:ch],
                                    CSHt[64:114, cl:ch], op=ALU.subtract)
            # softmax 1
            nc.scalar.activation(g[:, cl:ch], E2[:, cl:ch], AF.Exp)
            nc.vector.tensor_reduce(Z[:, lo:hi], gview(g, lo, hi),
                                    axis=mybir.AxisListType.X, op=ALU.add)
            nc.vector.reciprocal(Zr[:, lo:hi], Z[:, lo:hi])
            nc.gpsimd.tensor_tensor(gview(att1, lo, hi), gview(g, lo, hi),
                                    bview(Zr, lo, hi), op=ALU.mult)
            # softmax 2
            nc.scalar.activation(g2[:, cl:ch], att1[:, cl:ch], AF.Exp)
            nc.vector.tensor_reduce(Z2[:, lo:hi], gview(g2, lo, hi),
                                    axis=mybir.AxisListType.X, op=ALU.add)
            nc.vector.reciprocal(Z2r[:, lo:hi], Z2[:, lo:hi])
            # final scale, writing transposed back to n1-major for output
            oswap = bass.AP(tensor=outF.tensor, offset=outF.offset + lo,
                            ap=[[FO, 114], [1, hi - lo], [N, N]])
            nc.gpsimd.tensor_tensor(oswap, gview(g2, lo, hi),
                                    bview(Z2r, lo, hi), op=ALU.mult)

            # outputs for this n2-group: cols {n1*25 + g} of rows (b, t)/(b, j)
            for (p0, od, eng) in ((0, outs_d, nc.sync), (64, outt_d, nc.scalar)):
                srcv = bass.AP(tensor=outF.tensor,
                               offset=outF.offset + p0 * FO + lo,
                               ap=[[FO, 50], [N, N], [1, hi - lo]])
                dstv = bass.AP(tensor=od.tensor, offset=od.offset + lo,
                               ap=[[NN, 50], [N, N], [1, hi - lo]])
                eng.dma_start(dstv, srcv)

    nc.compile()
    return nc


_PROGRAM = None


def _get_program():
    global _PROGRAM
    if _PROGRAM is None:
        _PROGRAM = _build_program()
    return _PROGRAM


# ------------------------------------------------------------------ kernel --

def kernel(src, W_s, a_s, W_t, a_t):
    from concourse.bass_utils import run_bass_kernel_spmd

    src = np.ascontiguousarray(np.asarray(src, dtype=np.float32))
    wa4, qstk_s, qstk_t, csh = _host_consts(np.asarray(W_s), np.asarray(a_s),
                                            np.asarray(W_t), np.asarray(a_t))
    nc = _get_program()
    in_maps = []
    for c in range(NCORES):
        in_maps.append({
            "src_l": src[c * BL:(c + 1) * BL],
            "wa4": wa4, "qstk_s": qstk_s, "qstk_t": qstk_t, "csh": csh,
        })
    res = run_bass_kernel_spmd(nc, in_maps, core_ids=list(range(NCORES)))
    out_s = np.concatenate([res.results[c]["out_s"] for c in range(NCORES)], axis=0)
    out_t = np.concatenate([res.results[c]["out_t"] for c in range(NCORES)], axis=0)
    return out_s, out_t
